# revision 35
# baseline (speedup 1.0000x reference)
"""MHNA (masked, exp(n)-normalized multi-head attention) Trainium2 Bass kernel.

Sharding: 8 cores = batch(2) x head-groups(4 heads each). Core c (b=c//4,
g=c%4) receives the FULL (transposed, bf16) x of its batch plus its 4 heads'
weight slices and the full Wo. On device: project Q/K/V/n for the 4 heads over
all 2048 tokens, compute the 4 heads' masked-normalized attention, then
exchange context through TWO 8-core AllToAlls (one per head-pair, so the
first exchange hides under the second pair's compute) so that core r ends up
with the all-16-head context for token slice [256r, 256r+256) of BOTH
batches (batch b's ctx lands in A2A blocks 4b..4b+3). Each core finishes
with the out-projection (full Wo, +bo) for its 2x256 rows. No AllGather (x
arrives as a device-resident operand) and no ReduceScatter (the A2As move
~1MB total instead of ~4MB and the out-projection happens after the
exchange).

The program is core-uniform: every core sends its ctx t-slice r to A2A block
r, so no partition-id-dependent addressing is needed. Host assembly:
out[b, 256r:256r+256] = core_r_out[256b:256b+256].

Device layout choices (validated against a numpy mirror):
  - x arrives pre-transposed (xT columns) so every projection streams with the
    contraction dim (D) on partitions.
  - Q/K are produced transposed (QT/KT = W.T @ xT) in head-pair tiles [128, S]:
    partitions 0:64 = even head, 64:128 = odd head. Scores run as K=64
    row-packed matmuls (two heads concurrently in the PE array).
  - The causal mask and the exp(n) normalizer: scores*mask/exp(n_t). The
    normalizer is folded into Q (q_t scaled by exp(-n_t) before scores); the
    mask is a single sliding-window Const tile maskB[128, 896] applied during
    the PSUM->SBUF evacuation of diagonal score blocks.
  - ctx is produced transposed (ctxT = V.T @ ST); its [128, 512] tiles DMA
    straight into the A2A input (rows = dv, cols = t) with no transposes.
    The post-A2A layout (dv on rows, head order preserved) is exactly the
    lhsT the out-projection needs.
  - All SBUF intermediates are bf16 (halves DVE evacuation cost vs fp32).
  - Biases ride for free where possible: bn and bk are per-partition ACT
    biases on the exp()/kt evacuations; bq/bv/bo are K=1 rank-1 matmuls
    accumulated into the projection PSUMs.

Scheduling (tuned against the Tile scheduler's cost-model trace):
  - DMA issue order: xt(tg0) first (it gates the first matmul); the 2MB wo
    load is deferred into stage-1 issuance since it is first read after the
    A2As. Diagonal score blocks shorten matmul/evac/ctx to the live
    N=512-128r columns. K is projected before the wrep/Q chain so the PE
    never waits on the exp() activation. The post-A2A context is staged in
    per-256-row tiles (fine-grained deps), and the out-projection issues its
    bias + pair-0 matmuls first with PSUM accumulators alternating between
    two pools (5 banks) so they prefill under the second A2A.
"""
import numpy as np
import ml_dtypes

import concourse.bacc as bacc
import concourse.mybir as mybir
import concourse.tile as tile
from concourse.bass_utils import run_bass_kernel_spmd

F32 = mybir.dt.float32
BF16 = mybir.dt.bfloat16
AF = mybir.ActivationFunctionType
ALU = mybir.AluOpType
NPBF16 = ml_dtypes.bfloat16

B, S, D, H, DH = 2, 2048, 1024, 16, 64
HL = 4            # heads per core
NTG = 4           # t groups of 512
NTC = 16          # t chunks of 128

# wx columns: [ xT | wq | wk | wv | wn | Wo | bias | bo ]
_XC = S                      # 2048
_WQ = _XC                    # 2048
_WK = _WQ + 256              # 2304
_WV = _WK + 256              # 2560
_WN = _WV + 256              # 2816
_WO = _WN + 4                # 2820
_BIAS = _WO + D              # 3844
_NCOL = _BIAS + 2            # 3846
_IN_SPECS = dict(
    wx=((D, _NCOL), BF16),
)

RG8 = [[0, 1, 2, 3, 4, 5, 6, 7]]


def _kernel_body(tc, out, ins, mask_dram, sel_dram, ones_dram, phases=(1, 2, 3)):
    nc = tc.nc
    with (
        tc.tile_pool(name="dram", bufs=1, space="DRAM") as dp,
        tc.tile_pool(name="const", bufs=1) as cp,
        tc.tile_pool(name="xtp", bufs=3) as xtp,
        tc.tile_pool(name="big", bufs=1) as bigp,
        tc.tile_pool(name="stp", bufs=12) as stp,
        tc.tile_pool(name="outp", bufs=3) as outp,
        tc.tile_pool(name="ps_st", bufs=3, space="PSUM") as ps_st,
        tc.tile_pool(name="ps_ctx", bufs=2, space="PSUM") as ps_ctx,
        tc.tile_pool(name="ps_gen", bufs=2, space="PSUM") as ps_gen,
        tc.tile_pool(name="ps_vn", bufs=1, space="PSUM") as ps_vn,
    ):
        # one A2A per head-pair: the pair-0 exchange hides under pair-1 compute
        a2a_in = [dp.tile([8 * 128, 256], BF16, name=f"a2ai{p}") for p in range(2)]
        a2a_out = [dp.tile([8 * 128, 256], BF16, name=f"a2ao{p}") for p in range(2)]

        # ---- constants / weights to SBUF ----
        # The sync queue issues DMAs in program order: load the first x tile
        # before anything else (it gates the first matmul), and defer the
        # 2MB wo load (needed only after the A2A) until stage-1 issuance.
        xt_tiles = {}

        def load_xt(tg):
            xt_tiles[tg] = xtp.tile([128, 8, 512], BF16, tag="xt",
                                    name=f"xt{tg}")
            nc.sync.dma_start(
                xt_tiles[tg][:],
                ins["wx"][:, tg * 512:(tg + 1) * 512].rearrange(
                    "(a p) t -> p a t", p=128))

        if 1 in phases:
            load_xt(0)
        # separate tiles per weight: Tile deps are whole-tile, so the first
        # matmuls (n-proj, K) only wait for their own small loads, ordered
        # by first use.
        wn_sb = cp.tile([128, 8, 4], BF16)
        wk_sb = cp.tile([128, 8, 256], BF16)
        wq_sb = cp.tile([128, 8, 256], BF16)
        wv_sb = cp.tile([128, 8, 256], BF16)
        wo_sb = cp.tile([128, 8, D], BF16)
        mask_sb = cp.tile([128, 896], F32)
        sel_sb = cp.tile([HL, 256], BF16)
        ones512_sb = cp.tile([1, 512], BF16)
        bias_sb = cp.tile([1, D], BF16)
        bo_sb = cp.tile([1, D], BF16)
        bnc_sb = cp.tile([HL, 1], BF16)       # -bn: exp() ACT bias column
        bkc_sb = cp.tile([128, 2, 1], BF16)   # bk: kt-evac ACT bias columns
        nc.sync.dma_start(
            wn_sb[:], ins["wx"][:, _WN:_WO].rearrange("(a p) c -> p a c", p=128))
        nc.sync.dma_start(
            wk_sb[:], ins["wx"][:, _WK:_WV].rearrange("(a p) c -> p a c", p=128))
        nc.sync.dma_start(sel_sb[:], sel_dram[:])
        nc.sync.dma_start(bnc_sb[:], ins["wx"][768:772, _BIAS:_BIAS + 1])
        nc.sync.dma_start(bias_sb[:],
                          ins["wx"][:, _BIAS:_BIAS + 1].rearrange("d c -> c d"))
        nc.sync.dma_start(
            bkc_sb[:],
            ins["wx"][256:512, _BIAS:_BIAS + 1].rearrange("(a p) c -> p a c",
                                                          p=128))
        nc.sync.dma_start(ones512_sb[:], ones_dram[:])
        nc.sync.dma_start(
            wq_sb[:], ins["wx"][:, _WQ:_WK].rearrange("(a p) c -> p a c", p=128))
        nc.sync.dma_start(
            wv_sb[:], ins["wx"][:, _WV:_WN].rearrange("(a p) c -> p a c", p=128))
        nc.sync.dma_start(mask_sb[:], mask_dram[:])
        nc.sync.dma_start(bo_sb[:],
                          ins["wx"][:, _BIAS + 1:_BIAS + 2].rearrange("d c -> c d"))
        ones_sb = ones512_sb[0:1, 0:128]
        bq_row = [bias_sb[0:1, 128 * p:128 * p + 128] for p in range(2)]
        bvr_sb = bias_sb[0:1, 512:768]

        qt_sb = bigp.tile([128, 2, S], BF16)      # [part, pair, t]
        kt_sb = bigp.tile([128, 2, S], BF16)
        v_sb = bigp.tile([128, NTC, 256], BF16)   # [s-in-chunk, chunk, hc]
        wt_sb = bigp.tile([HL, S], BF16)          # exp(-(n+bn)) per local head

        # ================= stage 1: projections =================
        for tg in range(NTG if 1 in phases else 0):
            tsl = slice(tg * 512, (tg + 1) * 512)
            if tg + 1 < NTG:
                load_xt(tg + 1)
            if tg == NTG - 1:
                # wo is first read after the A2A; its 2MB DMA rides under
                # stage 1/2 compute from here.
                nc.sync.dma_start(
                    wo_sb[:],
                    ins["wx"][:, _WO:_WO + D].rearrange("(a p) c -> p a c", p=128))
            xt_tg = xt_tiles[tg]

            # N-projection -> wT = exp(-(n_pre + bn)); -bn rides the ACT bias
            n_ps = ps_vn.tile([HL, 512], F32, tag="v")
            for dc in range(8):
                nc.tensor.matmul(n_ps[:], wn_sb[:, dc, :], xt_tg[:, dc, :],
                                 start=(dc == 0), stop=(dc == 7))
            nc.scalar.activation(wt_sb[:, tsl], n_ps[:], AF.Exp,
                                 scale=-1.0, bias=bnc_sb[:])

            for pair in range(2):
                psl = slice(128 * pair, 128 * pair + 128)
                # KT first: it doesn't need wrep, so the PE isn't stalled on
                # the exp() activation latency at the head of each tg.
                k_ps = ps_gen.tile([128, 512], F32, tag="gen")
                for dc in range(8):
                    nc.tensor.matmul(k_ps[:], wk_sb[:, dc, psl], xt_tg[:, dc, :],
                                     start=(dc == 0), stop=(dc == 7))
                nc.scalar.activation(kt_sb[:, pair, tsl], k_ps[:], AF.Identity,
                                     bias=bkc_sb[:, pair, :])
                # wrep[p, t] = exp(-n) broadcast: partitions 0:64 <- even head
                wrep_ps = ps_gen.tile([128, 512], F32, tag="gen")
                nc.tensor.matmul(wrep_ps[:], sel_sb[:, psl], wt_sb[:, tsl],
                                 start=True, stop=True)
                wrep_sb = outp.tile([128, 512], F32, tag="wrep_sb")
                nc.scalar.copy(wrep_sb[:], wrep_ps[:])
                # QT
                q_ps = ps_gen.tile([128, 512], F32, tag="gen")
                for dc in range(8):
                    nc.tensor.matmul(q_ps[:], wq_sb[:, dc, psl], xt_tg[:, dc, :],
                                     start=(dc == 0), stop=False)
                nc.tensor.matmul(q_ps[:], bq_row[pair], ones512_sb,
                                 start=False, stop=True)
                nc.vector.tensor_mul(qt_sb[:, pair, tsl], q_ps[:], wrep_sb[:])

            # V (+bias via rank-1 matmul)
            for tl in range(4):
                tc16 = tg * 4 + tl
                v_ps = ps_vn.tile([128, 256], F32, tag="v")
                for dc in range(8):
                    nc.tensor.matmul(v_ps[:], xt_tg[:, dc, tl * 128:(tl + 1) * 128],
                                     wv_sb[:, dc, :], start=(dc == 0), stop=False)
                nc.tensor.matmul(v_ps[:], ones_sb[:], bvr_sb[:],
                                 start=False, stop=True)
                if tl % 2 == 0:
                    nc.vector.tensor_copy(v_sb[:, tc16, :], v_ps[:])
                else:
                    nc.scalar.copy(v_sb[:, tc16, :], v_ps[:])

        # ================= stage 2: scores + ctx -> A2A input =================
        # cxt chunk (pp, i%4) of batch i//4 multiplies Wo rows
        # [256*(i%4) + 128*pp ...] (head order). Loaded per bb-half so the
        # pair-0 half rides right behind its A2A and the out-projection can
        # start on batch 0 before batch 1 lands.
        cxt_sb = [[bigp.tile([128, 2, 256], BF16, name=f"cxt{p}_{q}")
                   for q in range(4)] for p in range(2)]

        def load_cxt(pp):
            for q in range(4):
                nc.sync.dma_start(
                    cxt_sb[pp][q][:],
                    a2a_out[pp][256 * q:256 * q + 256, :].rearrange(
                        "(a p) t -> p a t", p=128))

        ndve = 0
        for pair in range(2 if 2 in phases else 0):
            for tg in range(NTG):
                tsl = slice(tg * 512, (tg + 1) * 512)
                ctx_ps = [ps_ctx.tile([64, 512], F32, tag="ctx", name=f"ctx{_h}")
                          for _h in range(2)]
                nblk = 4 * tg + 4
                # diagonal blocks (r>=1) only touch queries q >= 128r: shorten
                # the score matmul, the masked evacuation, and the ctx matmul
                # to the live N = 512-128r columns.
                prev_sb, prev_j, prev_off = None, -1, 0
                for j in range(nblk):
                    r = j - 4 * tg
                    qoff = 128 * r if r > 0 else 0
                    nr = 512 - qoff
                    st_list = []
                    for hh in range(2):
                        hsl = slice(64 * hh, 64 * hh + 64)
                        st_ps = ps_st.tile([128, 512], F32, tag="st")
                        nc.tensor.matmul(
                            st_ps[:, 0:nr], kt_sb[hsl, pair, j * 128:(j + 1) * 128],
                            qt_sb[hsl, pair, tg * 512 + qoff:(tg + 1) * 512],
                            start=True, stop=True,
                            tile_position=(64 * hh, 0))
                        st_list.append(st_ps)
                    cur_sb = []
                    for hh in range(2):
                        st_sb = stp.tile([128, 512], BF16, tag="st_sb")
                        if r >= 0:
                            nc.vector.tensor_mul(
                                st_sb[:, 0:nr], st_list[hh][:, 0:nr],
                                mask_sb[:, 384:896 - qoff])
                        else:
                            ndve += 1
                            if ndve % 3 == 0:
                                nc.vector.tensor_copy(st_sb[:], st_list[hh][:])
                            else:
                                nc.scalar.copy(st_sb[:], st_list[hh][:])
                        cur_sb.append(st_sb)
                    if prev_sb is not None:
                        for hh in range(2):
                            hl_g = 2 * pair + hh
                            nc.tensor.matmul(
                                ctx_ps[hh][:, prev_off:512],
                                v_sb[:, prev_j, 64 * hl_g:64 * hl_g + 64],
                                prev_sb[hh][:, 0:512 - prev_off],
                                start=(prev_j == 0), stop=False)
                    prev_sb, prev_j, prev_off = cur_sb, j, qoff
                for hh in range(2):
                    hl_g = 2 * pair + hh
                    nc.tensor.matmul(
                        ctx_ps[hh][:, prev_off:512],
                        v_sb[:, prev_j, 64 * hl_g:64 * hl_g + 64],
                        prev_sb[hh][:, 0:512 - prev_off],
                        start=(prev_j == 0), stop=True)
                ctxt_sb = stp.tile([128, 512], BF16, tag="ctxt_sb")
                for hh in range(2):
                    if (tg + hh) % 2 == 0:
                        nc.vector.tensor_copy(ctxt_sb[64*hh:64*hh+64, :], ctx_ps[hh][:])
                    else:
                        nc.scalar.copy(ctxt_sb[64*hh:64*hh+64, :], ctx_ps[hh][:])
                # ctx t-slice r goes to A2A block r (rows 128r+p of pair's A2A)
                for half in range(2):
                    r = 2 * tg + half
                    nc.sync.dma_start(
                        a2a_in[pair][128 * r:128 * r + 128, :],
                        ctxt_sb[:, 256 * half:256 * half + 256])
            if 3 in phases:
                nc.gpsimd.collective_compute(
                    "AllToAll", ALU.bypass, replica_groups=RG8,
                    ins=[a2a_in[pair].opt()], outs=[a2a_out[pair].opt()])
                load_cxt(pair)

        # ================= stage 3: out projection =================
        if 3 in phases:
            # PE is in-order: to actually run work under the second A2A, the
            # pair-0-only accumulations (bias + even wo rows, fed by the
            # first A2A) must be ISSUED before any pair-1-dependent matmul.
            # Phase A runs bias+pair-0 for as many groups as there are free
            # PSUM banks (2 gen + 3 st); phase B completes them and frees
            # the banks for the remaining groups.
            groups = [(bb, tch, eb)
                      for bb in range(2) for tch in range(2) for eb in range(2)]
            pool_for = [ps_gen, ps_st, ps_st, ps_gen, ps_st,
                        ps_gen, ps_st, ps_gen]
            tag_for = ["gen", "st", "st", "gen", "st", "gen", "st", "gen"]
            o_tiles, out_tiles = {}, {}

            def phase_a(i):
                bb, tch, eb = groups[i]
                if (bb, tch) not in out_tiles:
                    out_tiles[(bb, tch)] = outp.tile([128, D], BF16, tag="out",
                                                     name="out_sb")
                esl = slice(eb * 512, (eb + 1) * 512)
                csl = slice(128 * tch, 128 * tch + 128)
                o_ps = pool_for[i].tile([128, 512], F32, tag=tag_for[i],
                                        name="o_ps")
                o_tiles[i] = o_ps
                nc.tensor.matmul(o_ps[:], ones_sb[:], bo_sb[:, esl],
                                 start=True, stop=False)
                for gi in range(4):
                    a = 4 * bb + gi
                    nc.tensor.matmul(
                        o_ps[:], cxt_sb[0][a // 2][:, a % 2, csl],
                        wo_sb[:, 2 * gi, esl], start=False, stop=False)

            def phase_b(i):
                bb, tch, eb = groups[i]
                esl = slice(eb * 512, (eb + 1) * 512)
                csl = slice(128 * tch, 128 * tch + 128)
                o_ps = o_tiles[i]
                for gi in range(4):
                    a = 4 * bb + gi
                    nc.tensor.matmul(
                        o_ps[:], cxt_sb[1][a // 2][:, a % 2, csl],
                        wo_sb[:, 2 * gi + 1, esl], start=False, stop=(gi == 3))
                out_sb = out_tiles[(bb, tch)]
                if eb == 0:
                    nc.vector.tensor_copy(out_sb[:, esl], o_ps[:])
                else:
                    nc.scalar.copy(out_sb[:, esl], o_ps[:])
                    nc.sync.dma_start(
                        out[256 * bb + 128 * tch:256 * bb + 128 * tch + 128, :],
                        out_sb[:])

            for i in range(5):
                phase_a(i)
            for i in range(5):
                phase_b(i)
            for i in range(5, 8):
                phase_a(i)
                phase_b(i)


def build_nc(phases=(1, 2, 3)):
    nc = bacc.Bacc("TRN2", target_bir_lowering=False, debug=False, num_devices=8,
                   enable_partition_id=False)
    ins = {k: nc.dram_tensor(k, list(s), dt, kind="ExternalInput").ap()
           for k, (s, dt) in _IN_SPECS.items()}
    out = nc.dram_tensor("out", [512, D], BF16, kind="ExternalOutput").ap()
    mask_dram = nc.inline_tensor(_make_maskB(), name="maskB").ap()
    sel = np.zeros((4, 256), dtype=NPBF16)
    for p in range(2):
        sel[2 * p + 0, 128 * p:128 * p + 64] = 1.0
        sel[2 * p + 1, 128 * p + 64:128 * p + 128] = 1.0
    sel_dram = nc.inline_tensor(sel, name="selc").ap()
    ones_dram = nc.inline_tensor(np.ones((1, 512), dtype=NPBF16), name="onesc").ap()
    with tile.TileContext(nc) as tc:
        _kernel_body(tc, out, ins, mask_dram, sel_dram, ones_dram, phases=phases)
    nc.compile()
    return nc


def _make_maskB():
    m = np.zeros((128, 896), dtype=np.float32)
    s = np.arange(128)[:, None]
    c = np.arange(896)[None, :]
    m[(c >= 384) & ((c - 384) >= s)] = 1.0
    m[:, 512:] = 1.0
    return m


def core_inputs(inp, c):
    b, hg = c // 4, c % 4
    heads = list(range(4 * hg, 4 * hg + 4))
    x = np.asarray(inp["x"], dtype=np.float32)
    Wqk = np.asarray(inp["Wqk"], dtype=np.float32)
    bqk = np.asarray(inp["bqk"], dtype=np.float32)
    Wv = np.asarray(inp["Wv"], dtype=np.float32)
    bv = np.asarray(inp["bv"], dtype=np.float32)
    Wn = np.asarray(inp["Wn"], dtype=np.float32)
    bn = np.asarray(inp["bn"], dtype=np.float32)
    Wo = np.asarray(inp["Wo"], dtype=np.float32)
    bo = np.asarray(inp["bo"], dtype=np.float32)
    wx = np.empty((D, _NCOL), dtype=NPBF16)
    wx[:, 0:S] = x[b].T
    wx[:, _WQ:_WQ + 256] = np.concatenate(
        [Wqk[:, h * 64:(h + 1) * 64] for h in heads], 1)
    wx[:, _WK:_WK + 256] = np.concatenate(
        [Wqk[:, 1024 + h * 64:1024 + (h + 1) * 64] for h in heads], 1)
    wx[:, _WV:_WV + 256] = np.concatenate(
        [Wv[:, h * 64:(h + 1) * 64] for h in heads], 1)
    wx[:, _WN:_WN + 4] = Wn[:, heads]
    wx[:, _WO:_WO + D] = Wo
    bias_col = np.zeros(D, dtype=np.float32)
    bias_col[0:256] = np.concatenate([bqk[h * 64:(h + 1) * 64] for h in heads])
    bias_col[256:512] = np.concatenate(
        [bqk[1024 + h * 64:1024 + (h + 1) * 64] for h in heads])
    bias_col[512:768] = np.concatenate([bv[h * 64:(h + 1) * 64] for h in heads])
    bias_col[768:772] = -bn[heads]          # exp() ACT bias wants -bn
    wx[:, _BIAS] = bias_col
    wx[:, _BIAS + 1] = bo
    return {"wx": np.ascontiguousarray(wx)}


_NC_CACHE = {}


def _get_nc():
    if "nc" not in _NC_CACHE:
        _NC_CACHE["nc"] = build_nc()
    return _NC_CACHE["nc"]


def _run(inputs, **spmd_kwargs):
    nc = _get_nc()
    in_maps = [core_inputs(inputs, c) for c in range(8)]
    # The tunneled device pool occasionally drops an execution (mesh
    # desync / worker hangup); a fresh attempt usually goes through.
    for attempt in range(3):
        try:
            res = run_bass_kernel_spmd(nc, in_maps, list(range(8)), **spmd_kwargs)
            break
        except Exception:
            if attempt == 2:
                raise
            import time
            time.sleep(10)
    out = np.empty((B, S, D), dtype=np.float32)
    for r in range(8):
        o = np.asarray(res.results[r]["out"], dtype=np.float32)
        out[0, 256 * r:256 * r + 256, :] = o[0:256]
        out[1, 256 * r:256 * r + 256, :] = o[256:512]
    return out, res


def kernel(**inputs):
    out, _ = _run(inputs)
    return out


# revision 37
# speedup vs baseline: 1.0367x; 1.0367x over previous
"""MHNA (masked, exp(n)-normalized multi-head attention) Trainium2 Bass kernel.

Sharding: 8 cores = batch(2) x head-groups(4 heads each). Core c (b=c//4,
g=c%4) receives the FULL (transposed, bf16) x of its batch plus its 4 heads'
weight slices and the full Wo. On device: project Q/K/V/n for the 4 heads over
all 2048 tokens, compute the 4 heads' masked-normalized attention, then
exchange context through TWO 8-core AllToAlls (one per head-pair, so the
first exchange hides under the second pair's compute) so that core r ends up
with the all-16-head context for token slice [256r, 256r+256) of BOTH
batches (batch b's ctx lands in A2A blocks 4b..4b+3). Each core finishes
with the out-projection (full Wo, +bo) for its 2x256 rows. No AllGather (x
arrives as a device-resident operand) and no ReduceScatter (the A2As move
~1MB total instead of ~4MB and the out-projection happens after the
exchange).

The program is core-uniform: every core sends its ctx t-slice r to A2A block
r, so no partition-id-dependent addressing is needed. Host assembly:
out[b, 256r:256r+256] = core_r_out[256b:256b+256].

Device layout choices (validated against a numpy mirror):
  - x arrives pre-transposed (xT columns) so every projection streams with the
    contraction dim (D) on partitions.
  - Q/K are produced transposed (QT/KT = W.T @ xT) in head-pair tiles [128, S]:
    partitions 0:64 = even head, 64:128 = odd head. Scores run as K=64
    row-packed matmuls (two heads concurrently in the PE array).
  - The causal mask and the exp(n) normalizer: scores*mask/exp(n_t). The
    normalizer is folded into Q (q_t scaled by exp(-n_t) before scores); the
    mask is a single sliding-window Const tile maskB[128, 896] applied during
    the PSUM->SBUF evacuation of diagonal score blocks.
  - ctx is produced transposed (ctxT = V.T @ ST); its [128, 512] tiles DMA
    straight into the A2A input (rows = dv, cols = t) with no transposes.
    The post-A2A layout (dv on rows, head order preserved) is exactly the
    lhsT the out-projection needs.
  - All SBUF intermediates are bf16 (halves DVE evacuation cost vs fp32).
  - Biases ride for free where possible: bn and bk are per-partition ACT
    biases on the exp()/kt evacuations; bq/bv/bo are K=1 rank-1 matmuls
    accumulated into the projection PSUMs.

Scheduling (tuned against the Tile scheduler's cost-model trace):
  - DMA issue order: xt(tg0) first (it gates the first matmul); the 2MB wo
    load is deferred into stage-1 issuance since it is first read after the
    A2As. Diagonal score blocks shorten matmul/evac/ctx to the live
    N=512-128r columns. K is projected before the wrep/Q chain so the PE
    never waits on the exp() activation. The post-A2A context is staged in
    per-256-row tiles (fine-grained deps), and the out-projection issues its
    bias + pair-0 matmuls first with PSUM accumulators alternating between
    two pools (5 banks) so they prefill under the second A2A.
"""
import numpy as np
import ml_dtypes

import concourse.bacc as bacc
import concourse.mybir as mybir
import concourse.tile as tile
from concourse.bass_utils import run_bass_kernel_spmd

F32 = mybir.dt.float32
BF16 = mybir.dt.bfloat16
AF = mybir.ActivationFunctionType
ALU = mybir.AluOpType
NPBF16 = ml_dtypes.bfloat16

B, S, D, H, DH = 2, 2048, 1024, 16, 64
HL = 4            # heads per core
NTG = 4           # t groups of 512
NTC = 16          # t chunks of 128

# wx columns: [ xT | wq | wk | wv | wn | Wo | bias | bo ]
_XC = S                      # 2048
_WQ = _XC                    # 2048
_WK = _WQ + 256              # 2304
_WV = _WK + 256              # 2560
_WN = _WV + 256              # 2816
_WO = _WN + 4                # 2820
_BIAS = _WO + D              # 3844
_NCOL = _BIAS + 2            # 3846
_IN_SPECS = dict(
    wx=((D, _NCOL), BF16),
)

RG8 = [[0, 1, 2, 3, 4, 5, 6, 7]]


def _kernel_body(tc, out, ins, mask_dram, sel_dram, ones_dram, phases=(1, 2, 3)):
    nc = tc.nc
    with (
        tc.tile_pool(name="dram", bufs=1, space="DRAM") as dp,
        tc.tile_pool(name="const", bufs=1) as cp,
        tc.tile_pool(name="xtp", bufs=3) as xtp,
        tc.tile_pool(name="big", bufs=1) as bigp,
        tc.tile_pool(name="stp", bufs=12) as stp,
        tc.tile_pool(name="outp", bufs=3) as outp,
        tc.tile_pool(name="ps_st", bufs=3, space="PSUM") as ps_st,
        tc.tile_pool(name="ps_ctx", bufs=2, space="PSUM") as ps_ctx,
        tc.tile_pool(name="ps_gen", bufs=2, space="PSUM") as ps_gen,
        tc.tile_pool(name="ps_vn", bufs=1, space="PSUM") as ps_vn,
    ):
        # one A2A per head-pair: the pair-0 exchange hides under pair-1 compute
        a2a_in = [dp.tile([8 * 128, 256], BF16, name=f"a2ai{p}") for p in range(2)]
        a2a_out = [dp.tile([8 * 128, 256], BF16, name=f"a2ao{p}") for p in range(2)]

        # ---- constants / weights to SBUF ----
        # The sync queue issues DMAs in program order: load the first x tile
        # before anything else (it gates the first matmul), and defer the
        # 2MB wo load (needed only after the A2A) until stage-1 issuance.
        xt_tiles = {}

        def load_xt(tg):
            xt_tiles[tg] = xtp.tile([128, 8, 512], BF16, tag="xt",
                                    name=f"xt{tg}")
            nc.sync.dma_start(
                xt_tiles[tg][:],
                ins["wx"][:, tg * 512:(tg + 1) * 512].rearrange(
                    "(a p) t -> p a t", p=128))

        if 1 in phases:
            load_xt(0)
        # separate tiles per weight: Tile deps are whole-tile, so the first
        # matmuls (n-proj, K) only wait for their own small loads, ordered
        # by first use.
        wn_sb = cp.tile([128, 8, 4], BF16)
        wk_sb = cp.tile([128, 8, 256], BF16)
        wq_sb = cp.tile([128, 8, 256], BF16)
        wv_sb = cp.tile([128, 8, 256], BF16)
        wo_sb = cp.tile([128, 8, D], BF16)
        mask_sb = cp.tile([128, 896], F32)
        sel_sb = cp.tile([HL, 256], BF16)
        ones512_sb = cp.tile([1, 512], BF16)
        bias_sb = cp.tile([1, D], BF16)
        bo_sb = cp.tile([1, D], BF16)
        bnc_sb = cp.tile([HL, 1], BF16)       # -bn: exp() ACT bias column
        bkc_sb = cp.tile([128, 2, 1], BF16)   # bk: kt-evac ACT bias columns
        nc.sync.dma_start(
            wn_sb[:], ins["wx"][:, _WN:_WO].rearrange("(a p) c -> p a c", p=128))
        nc.sync.dma_start(
            wk_sb[:], ins["wx"][:, _WK:_WV].rearrange("(a p) c -> p a c", p=128))
        nc.sync.dma_start(sel_sb[:], sel_dram[:])
        nc.sync.dma_start(bnc_sb[:], ins["wx"][768:772, _BIAS:_BIAS + 1])
        nc.sync.dma_start(bias_sb[:],
                          ins["wx"][:, _BIAS:_BIAS + 1].rearrange("d c -> c d"))
        nc.sync.dma_start(
            bkc_sb[:],
            ins["wx"][256:512, _BIAS:_BIAS + 1].rearrange("(a p) c -> p a c",
                                                          p=128))
        nc.sync.dma_start(ones512_sb[:], ones_dram[:])
        nc.sync.dma_start(
            wq_sb[:], ins["wx"][:, _WQ:_WK].rearrange("(a p) c -> p a c", p=128))
        nc.sync.dma_start(
            wv_sb[:], ins["wx"][:, _WV:_WN].rearrange("(a p) c -> p a c", p=128))
        nc.sync.dma_start(mask_sb[:], mask_dram[:])
        nc.sync.dma_start(bo_sb[:],
                          ins["wx"][:, _BIAS + 1:_BIAS + 2].rearrange("d c -> c d"))
        ones_sb = ones512_sb[0:1, 0:128]
        bq_row = [bias_sb[0:1, 128 * p:128 * p + 128] for p in range(2)]
        bvr_sb = bias_sb[0:1, 512:768]

        qt_sb = bigp.tile([128, 2, S], BF16)      # [part, pair, t]
        kt_sb = bigp.tile([128, 2, S], BF16)
        v_sb = bigp.tile([128, NTC, 256], BF16)   # [s-in-chunk, chunk, hc]
        wt_sb = bigp.tile([HL, S], BF16)          # exp(-(n+bn)) per local head

        # ================= stage 1: projections =================
        for tg in range(NTG if 1 in phases else 0):
            tsl = slice(tg * 512, (tg + 1) * 512)
            if tg + 1 < NTG:
                load_xt(tg + 1)
            if tg == NTG - 1:
                # wo is first read after the A2A; its 2MB DMA rides under
                # stage 1/2 compute from here.
                nc.sync.dma_start(
                    wo_sb[:],
                    ins["wx"][:, _WO:_WO + D].rearrange("(a p) c -> p a c", p=128))
            xt_tg = xt_tiles[tg]

            # N-projection -> wT = exp(-(n_pre + bn)); -bn rides the ACT bias
            n_ps = ps_vn.tile([HL, 512], F32, tag="v")
            for dc in range(8):
                nc.tensor.matmul(n_ps[:], wn_sb[:, dc, :], xt_tg[:, dc, :],
                                 start=(dc == 0), stop=(dc == 7))
            nc.scalar.activation(wt_sb[:, tsl], n_ps[:], AF.Exp,
                                 scale=-1.0, bias=bnc_sb[:])

            for pair in range(2):
                psl = slice(128 * pair, 128 * pair + 128)
                # KT first: it doesn't need wrep, so the PE isn't stalled on
                # the exp() activation latency at the head of each tg.
                k_ps = ps_gen.tile([128, 512], F32, tag="gen")
                for dc in range(8):
                    nc.tensor.matmul(k_ps[:], wk_sb[:, dc, psl], xt_tg[:, dc, :],
                                     start=(dc == 0), stop=(dc == 7))
                nc.scalar.activation(kt_sb[:, pair, tsl], k_ps[:], AF.Identity,
                                     bias=bkc_sb[:, pair, :])
                # wrep[p, t] = exp(-n) broadcast: partitions 0:64 <- even head
                wrep_ps = ps_gen.tile([128, 512], F32, tag="gen")
                nc.tensor.matmul(wrep_ps[:], sel_sb[:, psl], wt_sb[:, tsl],
                                 start=True, stop=True)
                wrep_sb = outp.tile([128, 512], F32, tag="wrep_sb")
                nc.scalar.copy(wrep_sb[:], wrep_ps[:])
                # QT
                q_ps = ps_gen.tile([128, 512], F32, tag="gen")
                for dc in range(8):
                    nc.tensor.matmul(q_ps[:], wq_sb[:, dc, psl], xt_tg[:, dc, :],
                                     start=(dc == 0), stop=False)
                nc.tensor.matmul(q_ps[:], bq_row[pair], ones512_sb,
                                 start=False, stop=True)
                nc.vector.tensor_mul(qt_sb[:, pair, tsl], q_ps[:], wrep_sb[:])

            # V (+bias via rank-1 matmul)
            for tl in range(4):
                tc16 = tg * 4 + tl
                v_ps = ps_vn.tile([128, 256], F32, tag="v")
                for dc in range(8):
                    nc.tensor.matmul(v_ps[:], xt_tg[:, dc, tl * 128:(tl + 1) * 128],
                                     wv_sb[:, dc, :], start=(dc == 0), stop=False)
                nc.tensor.matmul(v_ps[:], ones_sb[:], bvr_sb[:],
                                 start=False, stop=True)
                if tl % 2 == 0:
                    nc.vector.tensor_copy(v_sb[:, tc16, :], v_ps[:])
                else:
                    nc.scalar.copy(v_sb[:, tc16, :], v_ps[:])

        # ================= stage 2: scores + ctx -> A2A input =================
        # cxt chunk (pp, i%4) of batch i//4 multiplies Wo rows
        # [256*(i%4) + 128*pp ...] (head order). Loaded per bb-half so the
        # pair-0 half rides right behind its A2A and the out-projection can
        # start on batch 0 before batch 1 lands.
        cxt_sb = [[bigp.tile([128, 2, 256], BF16, name=f"cxt{p}_{q}")
                   for q in range(4)] for p in range(2)]

        def load_cxt(pp):
            for q in range(4):
                nc.sync.dma_start(
                    cxt_sb[pp][q][:],
                    a2a_out[pp][256 * q:256 * q + 256, :].rearrange(
                        "(a p) t -> p a t", p=128))

        ndve = 0
        for pair in range(2 if 2 in phases else 0):
            for tg in range(NTG):
                tsl = slice(tg * 512, (tg + 1) * 512)
                ctx_ps = [ps_ctx.tile([64, 512], F32, tag="ctx", name=f"ctx{_h}")
                          for _h in range(2)]
                nblk = 4 * tg + 4
                # diagonal blocks (r>=1) only touch queries q >= 128r: shorten
                # the score matmul, the masked evacuation, and the ctx matmul
                # to the live N = 512-128r columns.
                prev_sb, prev_j, prev_off = None, -1, 0
                for j in range(nblk):
                    r = j - 4 * tg
                    qoff = 128 * r if r > 0 else 0
                    nr = 512 - qoff
                    st_list = []
                    for hh in range(2):
                        hsl = slice(64 * hh, 64 * hh + 64)
                        st_ps = ps_st.tile([128, 512], F32, tag="st")
                        nc.tensor.matmul(
                            st_ps[:, 0:nr], kt_sb[hsl, pair, j * 128:(j + 1) * 128],
                            qt_sb[hsl, pair, tg * 512 + qoff:(tg + 1) * 512],
                            start=True, stop=True,
                            tile_position=(64 * hh, 0))
                        st_list.append(st_ps)
                    cur_sb = []
                    for hh in range(2):
                        st_sb = stp.tile([128, 512], BF16, tag="st_sb")
                        if r >= 0:
                            nc.vector.tensor_mul(
                                st_sb[:, 0:nr], st_list[hh][:, 0:nr],
                                mask_sb[:, 384:896 - qoff])
                        else:
                            ndve += 1
                            if ndve % 3 == 0:
                                nc.vector.tensor_copy(st_sb[:], st_list[hh][:])
                            else:
                                nc.scalar.copy(st_sb[:], st_list[hh][:])
                        cur_sb.append(st_sb)
                    if prev_sb is not None:
                        for hh in range(2):
                            hl_g = 2 * pair + hh
                            nc.tensor.matmul(
                                ctx_ps[hh][:, prev_off:512],
                                v_sb[:, prev_j, 64 * hl_g:64 * hl_g + 64],
                                prev_sb[hh][:, 0:512 - prev_off],
                                start=(prev_j == 0), stop=False)
                    prev_sb, prev_j, prev_off = cur_sb, j, qoff
                for hh in range(2):
                    hl_g = 2 * pair + hh
                    nc.tensor.matmul(
                        ctx_ps[hh][:, prev_off:512],
                        v_sb[:, prev_j, 64 * hl_g:64 * hl_g + 64],
                        prev_sb[hh][:, 0:512 - prev_off],
                        start=(prev_j == 0), stop=True)
                ctxt_sb = stp.tile([128, 512], BF16, tag="ctxt_sb")
                for hh in range(2):
                    if (tg + hh) % 2 == 0:
                        nc.vector.tensor_copy(ctxt_sb[64*hh:64*hh+64, :], ctx_ps[hh][:])
                    else:
                        nc.scalar.copy(ctxt_sb[64*hh:64*hh+64, :], ctx_ps[hh][:])
                # ctx t-slice r goes to A2A block r (rows 128r+p of pair's A2A)
                for half in range(2):
                    r = 2 * tg + half
                    nc.sync.dma_start(
                        a2a_in[pair][128 * r:128 * r + 128, :],
                        ctxt_sb[:, 256 * half:256 * half + 256])
            if 3 in phases:
                nc.gpsimd.collective_compute(
                    "AllToAll", ALU.bypass, replica_groups=RG8,
                    ins=[a2a_in[pair].opt()], outs=[a2a_out[pair].opt()])
                load_cxt(pair)

        # ================= stage 3: out projection =================
        if 3 in phases:
            # PE is in-order: to actually run work under the second A2A, the
            # pair-0-only accumulations (bias + even wo rows, fed by the
            # first A2A) must be ISSUED before any pair-1-dependent matmul.
            # Phase A runs bias+pair-0 for as many groups as there are free
            # PSUM banks (2 gen + 3 st); phase B completes them and frees
            # the banks for the remaining groups.
            groups = [(bb, tch, eb)
                      for bb in range(2) for tch in range(2) for eb in range(2)]
            pool_for = [ps_gen, ps_st, ps_st, ps_gen, ps_st,
                        ps_gen, ps_st, ps_gen]
            tag_for = ["gen", "st", "st", "gen", "st", "gen", "st", "gen"]
            o_tiles, out_tiles = {}, {}

            def phase_a(i):
                bb, tch, eb = groups[i]
                if (bb, tch) not in out_tiles:
                    out_tiles[(bb, tch)] = outp.tile([128, D], BF16, tag="out",
                                                     name="out_sb")
                esl = slice(eb * 512, (eb + 1) * 512)
                csl = slice(128 * tch, 128 * tch + 128)
                o_ps = pool_for[i].tile([128, 512], F32, tag=tag_for[i],
                                        name="o_ps")
                o_tiles[i] = o_ps
                nc.tensor.matmul(o_ps[:], ones_sb[:], bo_sb[:, esl],
                                 start=True, stop=False)
                for gi in range(4):
                    a = 4 * bb + gi
                    nc.tensor.matmul(
                        o_ps[:], cxt_sb[0][a // 2][:, a % 2, csl],
                        wo_sb[:, 2 * gi, esl], start=False, stop=False)

            def phase_b(i):
                bb, tch, eb = groups[i]
                esl = slice(eb * 512, (eb + 1) * 512)
                csl = slice(128 * tch, 128 * tch + 128)
                o_ps = o_tiles[i]
                for gi in range(4):
                    a = 4 * bb + gi
                    nc.tensor.matmul(
                        o_ps[:], cxt_sb[1][a // 2][:, a % 2, csl],
                        wo_sb[:, 2 * gi + 1, esl], start=False, stop=(gi == 3))
                out_sb = out_tiles[(bb, tch)]
                if eb == 0:
                    nc.vector.tensor_copy(out_sb[:, esl], o_ps[:])
                else:
                    nc.scalar.copy(out_sb[:, esl], o_ps[:])
                    nc.sync.dma_start(
                        out[256 * bb + 128 * tch:256 * bb + 128 * tch + 128, :],
                        out_sb[:])

            for i in range(5):
                phase_a(i)
            for i in range(5):
                phase_b(i)
            for i in range(5, 8):
                phase_a(i)
                phase_b(i)


def build_nc(phases=(1, 2, 3)):
    nc = bacc.Bacc("TRN2", target_bir_lowering=False, debug=False, num_devices=8,
                   enable_partition_id=False)
    ins = {k: nc.dram_tensor(k, list(s), dt, kind="ExternalInput").ap()
           for k, (s, dt) in _IN_SPECS.items()}
    out = nc.dram_tensor("out", [512, D], BF16, kind="ExternalOutput").ap()
    mask_dram = nc.inline_tensor(_make_maskB(), name="maskB").ap()
    sel = np.zeros((4, 256), dtype=NPBF16)
    for p in range(2):
        sel[2 * p + 0, 128 * p:128 * p + 64] = 1.0
        sel[2 * p + 1, 128 * p + 64:128 * p + 128] = 1.0
    sel_dram = nc.inline_tensor(sel, name="selc").ap()
    ones_dram = nc.inline_tensor(np.ones((1, 512), dtype=NPBF16), name="onesc").ap()
    with tile.TileContext(nc) as tc:
        _kernel_body(tc, out, ins, mask_dram, sel_dram, ones_dram, phases=phases)
    nc.compile()
    return nc


def _make_maskB():
    m = np.zeros((128, 896), dtype=np.float32)
    s = np.arange(128)[:, None]
    c = np.arange(896)[None, :]
    m[(c >= 384) & ((c - 384) >= s)] = 1.0
    m[:, 512:] = 1.0
    return m


def core_inputs(inp, c):
    b, hg = c // 4, c % 4
    heads = list(range(4 * hg, 4 * hg + 4))
    x = np.asarray(inp["x"], dtype=np.float32)
    Wqk = np.asarray(inp["Wqk"], dtype=np.float32)
    bqk = np.asarray(inp["bqk"], dtype=np.float32)
    Wv = np.asarray(inp["Wv"], dtype=np.float32)
    bv = np.asarray(inp["bv"], dtype=np.float32)
    Wn = np.asarray(inp["Wn"], dtype=np.float32)
    bn = np.asarray(inp["bn"], dtype=np.float32)
    Wo = np.asarray(inp["Wo"], dtype=np.float32)
    bo = np.asarray(inp["bo"], dtype=np.float32)
    wx = np.empty((D, _NCOL), dtype=NPBF16)
    wx[:, 0:S] = x[b].T
    wx[:, _WQ:_WQ + 256] = np.concatenate(
        [Wqk[:, h * 64:(h + 1) * 64] for h in heads], 1)
    wx[:, _WK:_WK + 256] = np.concatenate(
        [Wqk[:, 1024 + h * 64:1024 + (h + 1) * 64] for h in heads], 1)
    wx[:, _WV:_WV + 256] = np.concatenate(
        [Wv[:, h * 64:(h + 1) * 64] for h in heads], 1)
    wx[:, _WN:_WN + 4] = Wn[:, heads]
    wx[:, _WO:_WO + D] = Wo
    bias_col = np.zeros(D, dtype=np.float32)
    bias_col[0:256] = np.concatenate([bqk[h * 64:(h + 1) * 64] for h in heads])
    bias_col[256:512] = np.concatenate(
        [bqk[1024 + h * 64:1024 + (h + 1) * 64] for h in heads])
    bias_col[512:768] = np.concatenate([bv[h * 64:(h + 1) * 64] for h in heads])
    bias_col[768:772] = -bn[heads]          # exp() ACT bias wants -bn
    wx[:, _BIAS] = bias_col
    wx[:, _BIAS + 1] = bo
    return {"wx": np.ascontiguousarray(wx)}


_NC_CACHE = {}


def _get_nc():
    if "nc" not in _NC_CACHE:
        _NC_CACHE["nc"] = build_nc()
    return _NC_CACHE["nc"]


def _run(inputs, **spmd_kwargs):
    nc = _get_nc()
    in_maps = [core_inputs(inputs, c) for c in range(8)]
    # The tunneled device pool occasionally drops an execution (mesh
    # desync / worker hangup); a fresh attempt usually goes through.
    for attempt in range(3):
        try:
            res = run_bass_kernel_spmd(nc, in_maps, list(range(8)), **spmd_kwargs)
            break
        except Exception:
            if attempt == 2:
                raise
            import time
            time.sleep(10)
    out = np.empty((B, S, D), dtype=np.float32)
    for r in range(8):
        o = np.asarray(res.results[r]["out"], dtype=np.float32)
        out[0, 256 * r:256 * r + 256, :] = o[0:256]
        out[1, 256 * r:256 * r + 256, :] = o[256:512]
    return out, res


def kernel(**inputs):
    out, _ = _run(inputs)
    return out
